# revision 27
# baseline (speedup 1.0000x reference)
"""Trainium2 Bass kernel for DigitConvolutionalModel (conv3x3 + 4-layer MLP).

Strategy:
  - The 3x3 'VALID' conv on 28x28 is a linear map 784->676, so it folds into
    the first linear layer on the host: W1eff[784,1024] = C @ W1.T. The device
    kernel is then a pure 4-layer MLP: relu(x@W1e+b1) -> relu(@W2.T+b2) ->
    relu(@W3.T+b3) -> @W4.T+b4.
  - Pure data parallelism: batch 16384 sharded 8x -> 2048 rows per core.
  - Feature-major layout on device: activations are [features, batch] so each
    layer is out = lhsT.T @ rhs with lhsT = W[in,out] tiles, rhs = h[in, batch].
    Host transposes x shards to [784, 2048]; output comes back [10, 2048].
  - bf16 matmul inputs, fp32 PSUM accumulation (bf16 moving operand allows
    N=1024 free dim -> 2-PSUM-bank outputs, halving matmul count).
  - b1/b2/b3 fused into the ScalarE relu (per-partition bias AP); b4 via a
    K=1 ones-row matmul (last layer has no relu to hang a bias on).
"""

import numpy as np
import ml_dtypes
from contextlib import ExitStack

import concourse.bass as bass
import concourse.mybir as mybir
import concourse.tile as tile
from concourse import bacc
from concourse.bass_utils import run_bass_kernel_spmd

F32 = mybir.dt.float32
BF16 = mybir.dt.bfloat16
AF = mybir.ActivationFunctionType

N_CORES = 8
B = 16384
BC = B // N_CORES          # 2048 rows per core
BT = 512                   # batch tile (free dim per matmul; ISA caps mm free dim at 512)
NBT = BC // BT
K1 = 784                   # 28*28 (conv folded into W1)
D1, D2, D3, D4 = 1024, 512, 256, 10

PS_BUFS = (8 * 2048) // (BT * 4)   # PSUM banks / banks-per-tile
KP = [128] * 6 + [16]              # layer-1 K-tile partition sizes
KORD = [6, 0, 1, 2, 3, 4, 5]       # k6 first: its DMA is tiny, PE starts early

NP_BF16 = ml_dtypes.bfloat16


def _build_nc():
    # Bacc (not plain Bass): its compile pipeline runs
    # generate_event_semaphores, which splits multi-wait instructions (e.g.
    # the kernel-tail drain) into EventSemaphore preludes — TRN2 allows at
    # most one sync wait per instruction.
    nc = bacc.Bacc(None)

    x_d = nc.dram_tensor("x", [K1, BC], BF16, kind="ExternalInput")
    w1_d = nc.dram_tensor("w1", [K1, D1], BF16, kind="ExternalInput")
    w2_d = nc.dram_tensor("w2", [D1, D2], BF16, kind="ExternalInput")
    w3_d = nc.dram_tensor("w3", [D2, D3], BF16, kind="ExternalInput")
    w4_d = nc.dram_tensor("w4", [D3, D4], BF16, kind="ExternalInput")
    b4_d = nc.dram_tensor("b4", [1, D4], BF16, kind="ExternalInput")
    ones_d = nc.dram_tensor("ones", [1, BT], BF16, kind="ExternalInput")
    # bias cols: 0-7 = b1 m-tiles, 8-11 = b2, 12-13 = b3
    bias_d = nc.dram_tensor("bias", [128, 14], F32, kind="ExternalInput")
    out_d = nc.dram_tensor("out", [D4, BC], F32, kind="ExternalOutput")

    with tile.TileContext(nc) as tc, ExitStack() as ctx:
        sb = ctx.enter_context(tc.tile_pool(name="sb", bufs=1))
        psum = ctx.enter_context(tc.tile_pool(name="psum", bufs=PS_BUFS, space="PSUM"))

        # ---------------- persistent SBUF tiles + DMAs ----------------
        # The sync-engine HWDGE issue stream is serial (~0.7us/DMA), so emit
        # in consumption order: (w1_k, x_k_bt0) pairs in KORD order (k=6 pair
        # is tiny -> first matmul starts as early as possible), then bias,
        # then x for later bts, then later-layer weights.
        xt = [[None] * NBT for _ in range(7)]
        w1t = [None] * 7
        ko = [128 * k for k in range(7)]
        for k in KORD:
            wt = sb.tile([KP[k], D1], BF16, tag=f"w1_{k}", name=f"w1_{k}")
            nc.sync.dma_start(out=wt[:], in_=w1_d[ko[k]:ko[k] + KP[k], :])
            w1t[k] = wt
            t = sb.tile([KP[k], BT], BF16, tag=f"x_{k}_0", name=f"x_{k}_0")
            nc.sync.dma_start(out=t[:], in_=x_d[ko[k]:ko[k] + KP[k], 0:BT])
            xt[k][0] = t

        bias_sb = sb.tile([128, 14], F32, tag="bias", name="bias_sb")
        nc.sync.dma_start(out=bias_sb[:], in_=bias_d[:])
        # ACT "bias probe": pre-observe the bias DMA on ScalarE so the real
        # relus (which also wait on the PE semaphore) keep a single sync wait.
        probe = sb.tile([128, 1], F32, tag="probe", name="probe")
        nc.scalar.activation(probe[:], bias_sb[:, 0:1], AF.Relu,
                             bias=bias_sb[:, 0:1])

        for bt in range(1, NBT):
            for k in KORD:
                t = sb.tile([KP[k], BT], BF16, tag=f"x_{k}_{bt}", name=f"x_{k}_{bt}")
                nc.sync.dma_start(
                    out=t[:], in_=x_d[ko[k]:ko[k] + KP[k], bt * BT:(bt + 1) * BT])
                xt[k][bt] = t

        w2t = []
        for k in range(8):
            t = sb.tile([128, D2], BF16, tag=f"w2_{k}", name=f"w2_{k}")
            nc.sync.dma_start(out=t[:], in_=w2_d[k * 128:(k + 1) * 128, :])
            w2t.append(t)
        w3t = []
        for k in range(4):
            t = sb.tile([128, D3], BF16, tag=f"w3_{k}", name=f"w3_{k}")
            nc.sync.dma_start(out=t[:], in_=w3_d[k * 128:(k + 1) * 128, :])
            w3t.append(t)
        w4t = []
        for k in range(2):
            t = sb.tile([128, D4], BF16, tag=f"w4_{k}", name=f"w4_{k}")
            nc.sync.dma_start(out=t[:], in_=w4_d[k * 128:(k + 1) * 128, :])
            w4t.append(t)
        b4t = sb.tile([1, D4], BF16, tag="b4", name="b4t")
        nc.sync.dma_start(out=b4t[:], in_=b4_d[:])
        # ones comes in as an input DMA (a DVE memset would add the DVE
        # semaphore to the drain and an extra wait on the consuming matmul)
        ones = sb.tile([1, BT], BF16, tag="ones", name="ones")
        nc.sync.dma_start(out=ones[:], in_=ones_d[:])

        # activations
        h1 = [[sb.tile([128, BT], BF16, tag=f"h1_{m}_{bt}", name=f"h1_{m}_{bt}")
               for bt in range(NBT)] for m in range(8)]
        h2 = [[sb.tile([128, BT], BF16, tag=f"h2_{m}_{bt}", name=f"h2_{m}_{bt}")
               for bt in range(NBT)] for m in range(4)]
        h3 = [[sb.tile([128, BT], BF16, tag=f"h3_{m}_{bt}", name=f"h3_{m}_{bt}")
               for bt in range(NBT)] for m in range(2)]
        outsb = sb.tile([D4, BC], F32, tag="o", name="o")

        # ---------------- PE warmup ----------------
        # The PE HAM clock gate starts at 1.2 GHz and only releases to
        # 2.4 GHz after ~3.4us of sustained activity. Real matmuls can't
        # start until the first DMAs land (~10us); burn garbage matmuls on an
        # uninitialized SBUF tile from ~7.2us (end of engine preamble) so the
        # HAM fires before/soon after real work begins.
        warm_sb = sb.tile([128, 256], BF16, tag="warm", name="warm_sb")
        nc.gpsimd.memset(warm_sb[:], 1.0)
        warm_ps = psum.tile([128, BT], F32, tag="ps", name="warm_ps")
        for _ in range(26):
            nc.tensor.matmul(warm_ps[:, 0:128], warm_sb[:, 0:128],
                             warm_sb[:, 128:256], start=True, stop=True)

        def l1_group(p, m, bt):
            for j, k in enumerate(KORD):
                nc.tensor.matmul(
                    p[:], w1t[k][:, m * 128:(m + 1) * 128], xt[k][bt][:],
                    start=(j == 0), stop=(j == 6),
                )

        # ---------------- layer 1: [784, BC] -> [1024, BC] ----------------
        # bt0 in k-outer half-passes (PS_BUFS interleaved PSUM groups): the PE
        # consumes each (w1_k, x_k) pair right behind its DMA arrival.
        for half in range(8 // PS_BUFS):
            ms = range(half * PS_BUFS, (half + 1) * PS_BUFS)
            ps0 = {m: psum.tile([128, BT], F32, tag="ps", name=f"ps1_{m}_0")
                   for m in ms}
            for j, k in enumerate(KORD):
                for m in ms:
                    nc.tensor.matmul(
                        ps0[m][:], w1t[k][:, m * 128:(m + 1) * 128], xt[k][0][:],
                        start=(j == 0), stop=(j == 6),
                    )
            for m in ms:
                nc.scalar.activation(h1[m][0][:], ps0[m][:], AF.Relu,
                                     bias=bias_sb[:, m:m + 1])

        # bt1..: m-outer / k-inner (one PSUM group at a time; relu overlaps)
        for bt in range(1, NBT):
            # PE observer: pre-observe the previous bt's first relu so the
            # first matmul of this bt doesn't need (psum-slot WAR + fresh x
            # DMA) = two waits on one instruction.
            nc.tensor.ldweights(h1[0][bt - 1][:, 0:128])
            for m in range(8):
                p = psum.tile([128, BT], F32, tag="ps", name=f"ps1_{m}_{bt}")
                l1_group(p, m, bt)
                nc.scalar.activation(h1[m][bt][:], p[:], AF.Relu,
                                     bias=bias_sb[:, m:m + 1])

        # ---------------- layer 2: [1024, BC] -> [512, BC] ----------------
        for bt in range(NBT):
            for m in range(4):
                p = psum.tile([128, BT], F32, tag="ps", name=f"ps2_{m}_{bt}")
                for k in range(8):
                    nc.tensor.matmul(
                        p[:], w2t[k][:, m * 128:(m + 1) * 128], h1[k][bt][:],
                        start=(k == 0), stop=(k == 7),
                    )
                nc.scalar.activation(h2[m][bt][:], p[:], AF.Relu,
                                     bias=bias_sb[:, 8 + m:9 + m])

        # ---------------- layer 3: [512, BC] -> [256, BC] ----------------
        for bt in range(NBT):
            for m in range(2):
                p = psum.tile([128, BT], F32, tag="ps", name=f"ps3_{m}_{bt}")
                for k in range(4):
                    nc.tensor.matmul(
                        p[:], w3t[k][:, m * 128:(m + 1) * 128], h2[k][bt][:],
                        start=(k == 0), stop=(k == 3),
                    )
                nc.scalar.activation(h3[m][bt][:], p[:], AF.Relu,
                                     bias=bias_sb[:, 12 + m:13 + m])

        # ---------------- layer 4: [256, BC] -> [10, BC] (no relu) ----------------
        for bt in range(NBT):
            p = psum.tile([D4, BT], F32, tag="ps", name=f"ps4_{bt}")
            nc.tensor.matmul(p[:], w4t[0][:, :], h3[0][bt][:], start=True, stop=False)
            nc.tensor.matmul(p[:], w4t[1][:, :], h3[1][bt][:], start=False, stop=False)
            nc.tensor.matmul(p[:], b4t[:, :], ones[:, :], start=False, stop=True)
            nc.scalar.copy(outsb[:, bt * BT:(bt + 1) * BT], p[:])
            # per-bt output DMA on the (otherwise unused) SWDGE ring: earlier
            # bts stream out during compute, shortening the kernel tail.
            nc.gpsimd.dma_start(out=out_d[:, bt * BT:(bt + 1) * BT],
                                in_=outsb[:, bt * BT:(bt + 1) * BT])

    # run the Bacc pass pipeline (register alloc, wait splitting, ...);
    # run_bass_via_pjrt binds the primitive directly and never finalizes.
    nc.finalize()
    return nc


def _fold_conv(conv_w, W1):
    """W1eff[784,1024] such that x @ W1eff == conv3x3(x, conv_w) @ W1.T."""
    W1img = W1.reshape(D1, 26, 26).transpose(1, 2, 0).astype(np.float32)  # [26,26,1024]
    W1e = np.zeros((28, 28, D1), np.float32)
    for di in range(3):
        for dj in range(3):
            W1e[di:di + 26, dj:dj + 26, :] += np.float32(conv_w[di, dj]) * W1img
    return W1e.reshape(K1, D1)


def _prep_inputs(inputs):
    x = np.asarray(inputs["x"], np.float32)
    conv_w = np.asarray(inputs["conv_w"], np.float32)
    W1 = np.asarray(inputs["W1"], np.float32)
    b1 = np.asarray(inputs["b1"], np.float32)
    W2 = np.asarray(inputs["W2"], np.float32)
    b2 = np.asarray(inputs["b2"], np.float32)
    W3 = np.asarray(inputs["W3"], np.float32)
    b3 = np.asarray(inputs["b3"], np.float32)
    W4 = np.asarray(inputs["W4"], np.float32)
    b4 = np.asarray(inputs["b4"], np.float32)

    w1e = _fold_conv(conv_w, W1).astype(NP_BF16)                   # [784, 1024]
    w2 = np.ascontiguousarray(W2.T).astype(NP_BF16)                # [1024, 512]
    w3 = np.ascontiguousarray(W3.T).astype(NP_BF16)                # [512, 256]
    w4 = np.ascontiguousarray(W4.T).astype(NP_BF16)                # [256, 10]
    b4m = b4[None, :].astype(NP_BF16)                              # [1, 10]
    bias_pack = np.zeros((128, 14), np.float32)
    bias_pack[:, 0:8] = b1.reshape(8, 128).T
    bias_pack[:, 8:12] = b2.reshape(4, 128).T
    bias_pack[:, 12:14] = b3.reshape(2, 128).T

    shared = {"w1": w1e, "w2": w2, "w3": w3, "w4": w4, "b4": b4m,
              "bias": bias_pack, "ones": np.ones((1, BT), NP_BF16)}
    in_maps = []
    for c in range(N_CORES):
        xs = np.ascontiguousarray(x[c * BC:(c + 1) * BC].T).astype(NP_BF16)  # [784, 2048]
        in_maps.append({"x": xs, **shared})
    return in_maps


def _run(inputs, trace=False):
    nc = _build_nc()
    in_maps = _prep_inputs(inputs)
    res = run_bass_kernel_spmd(nc, in_maps, core_ids=list(range(N_CORES)),
                               trace=trace)
    parts = [np.asarray(r["out"], np.float32).T for r in res.results]  # [2048, 10] each
    out = np.concatenate(parts, axis=0)                                # [16384, 10]
    return out, res


def kernel(**inputs):
    out, _ = _run(inputs, trace=False)
    return out
